# revision 17
# baseline (speedup 1.0000x reference)
"""S5-style complex-diagonal SSM (MIMO layer) on 8 TRN2 NeuronCores.

Strategy (per core; batch-parallel, 2 of 16 batch elements per core):
  phase 0: load u (natural [t,h] tiles), PE-transpose to uT [h, t] in SBUF
  phase 1: Bu = u @ Bbar^T  (complex, via 2 real fp32r matmuls, PSUM out,
           layout [p partitions, t free])
  scan:    rotated-frame trick.  lam_p = r_p * e^{i th_p}:
             z_t  = e^{-i th t} (.) Bu_t          (DVE elementwise, tables)
             w_t  = r * w_{t-1} + z_t             (HW tensor_tensor_scan,
                                                   re/im decoupled real scans)
             x_t  = e^{+i th t} (.) w_t           (folded into phase 3)
  phase 3: y = 2 Re(C x) + D u:
             y = 2Cr@(c.wr) - 2Cr@(s.wi) - 2Ci@(c.wi) - 2Ci@(s.wr) + u@diag(D)
           (products c.wr etc. are 4 DVE mults; adds ride the PSUM
            accumulation of 10 fp32r matmuls per 128-token tile)
Output written in natural [t, h] layout -> contiguous DMA out.
"""

import os
import sys

import numpy as np
import ml_dtypes

BF16_NP = ml_dtypes.bfloat16

for _p in ("/opt/trn_rl_repo", "/root/.axon_site/_ro/trn_rl_repo"):
    if os.path.isdir(_p) and _p not in sys.path:
        sys.path.insert(0, _p)

import concourse.bass as bass
import concourse.mybir as mybir
import concourse.tile as tile
from concourse.bass_utils import run_bass_kernel_spmd

B_SZ, L, H, P = 16, 1024, 256, 256
NCORES = 8
BL = B_SZ // NCORES            # 2 batch elements per core
F32 = mybir.dt.float32
F32R = mybir.dt.float32r
BF16 = mybir.dt.bfloat16
AL = mybir.AluOpType

_PROG = None                   # cached compiled Bass program


def _emit(ctx, tc, nc, d_u, d_w1, d_cos, d_sin, d_r, d_cw, d_dd, d_y):
    consts = ctx.enter_context(tc.tile_pool(name="consts", bufs=1))

    # phase-1 weights: bbarT[ri][h, p] as lhsT tiles (k=h slice, m=p)
    w1 = [[consts.tile([128, P], BF16, name=f"w1_{ri}_{kh}")
          for kh in range(2)] for ri in range(2)]
    for ri in range(2):
        for kh in range(2):
            nc.sync.dma_start(w1[ri][kh], d_w1[ri, kh * 128:(kh + 1) * 128, :])

    # phase-3 weights cw[q][pt] (k=p slice, n=h) and diag(D) blocks
    cw = [[consts.tile([128, H], BF16, name=f"cw_{q}_{pt}")
          for pt in range(2)] for q in range(4)]
    for q in range(4):
        for pt in range(2):
            nc.sync.dma_start(cw[q][pt], d_cw[q, pt, :, :])
    dd = [consts.tile([128, H], BF16, name=f"dd_{kh}") for kh in range(2)]
    for kh in range(2):
        nc.sync.dma_start(dd[kh], d_dd[kh * 128:(kh + 1) * 128, :])

    # rotation tables (per p-tile) + decay broadcast
    ctab = [consts.tile([128, L], BF16, name=f"ctab_{pt}") for pt in range(2)]
    stab = [consts.tile([128, L], BF16, name=f"stab_{pt}") for pt in range(2)]
    rbc = [consts.tile([128, L], F32, name=f"rbc_{pt}") for pt in range(2)]
    for pt in range(2):
        nc.sync.dma_start(ctab[pt], d_cos[pt])
        nc.sync.dma_start(stab[pt], d_sin[pt])
        nc.sync.dma_start(rbc[pt], d_r[pt])

    # uT[kh]: u transposed, (128 h, BL*L tokens)
    ut_pool = ctx.enter_context(tc.tile_pool(name="ut", bufs=1))
    uT = [ut_pool.tile([128, BL * L], BF16, name=f"uT_{kh}") for kh in range(2)]

    for b in range(BL):
        for kh in range(2):
            nc.sync.dma_start(uT[kh][:, b * L:(b + 1) * L],
                              d_u[b, kh * 128:(kh + 1) * 128, :])

    bu_pool = ctx.enter_context(tc.tile_pool(name="bu", bufs=1, space="PSUM"))
    y_psum = ctx.enter_context(tc.tile_pool(name="yp", bufs=4, space="PSUM"))
    z_pool = ctx.enter_context(tc.tile_pool(name="z", bufs=2))
    t_pool = ctx.enter_context(tc.tile_pool(name="tmp", bufs=2))
    w_pool = ctx.enter_context(tc.tile_pool(name="w", bufs=2))
    p_pool = ctx.enter_context(tc.tile_pool(name="pp", bufs=2))
    y_sb = ctx.enter_context(tc.tile_pool(name="ysb", bufs=3))

    TT = nc.vector.tensor_tensor

    for b in range(BL):
        planes = {}
        for pt in range(2):
            c, s = ctab[pt], stab[pt]
            # ---- phase 1: Bu (PSUM) ----
            bus = []
            for ri in range(2):
                bu = bu_pool.tile([128, L], F32, tag=f"bu{ri}")
                for ns in range(2):
                    nsl = slice(ns * 512, (ns + 1) * 512)
                    for kh in range(2):
                        nc.tensor.matmul(
                            bu[:, nsl],
                            lhsT=w1[ri][kh][:, pt * 128:(pt + 1) * 128],
                            rhs=uT[kh][:, b * L + ns * 512:b * L + (ns + 1) * 512],
                            start=(kh == 0), stop=(kh == 1))
                bus.append(bu)
            # ---- pre-rotation: z = e^{-i th t} . Bu ----
            # ACT downcasts PSUM->SBUF bf16 (frees PSUM fast); DVE TTs then
            # run all-bf16 in 2x_1P mode
            br = z_pool.tile([128, L], BF16, tag="br")
            bi = z_pool.tile([128, L], BF16, tag="bi")
            nc.scalar.copy(br, bus[0])
            nc.scalar.copy(bi, bus[1])
            zr = z_pool.tile([128, L], BF16, tag="zr")
            zi = z_pool.tile([128, L], BF16, tag="zi")
            t1 = t_pool.tile([128, L], BF16, tag="t1")
            t2 = t_pool.tile([128, L], BF16, tag="t2")
            TT(t1, br, c, AL.mult)              # c*br
            TT(t2, bi, s, AL.mult)              # s*bi
            TT(zr, t1, t2, AL.add)
            t3 = t_pool.tile([128, L], BF16, tag="t3")
            t4 = t_pool.tile([128, L], BF16, tag="t4")
            TT(t3, bi, c, AL.mult)              # c*bi
            TT(t4, br, s, AL.mult)              # s*br
            TT(zi, t3, t4, AL.subtract)
            # ---- decay scans (HW linear recurrence along free dim) ----
            # data0 (decay r) stays fp32: coefficient error compounds over L
            wr = w_pool.tile([128, L], BF16, tag="wr")
            wi = w_pool.tile([128, L], BF16, tag="wi")
            nc.vector.tensor_tensor_scan(wr, rbc[pt], zr, 0.0, AL.mult, AL.add)
            nc.vector.tensor_tensor_scan(wi, rbc[pt], zi, 0.0, AL.mult, AL.add)
            # ---- post-rotation products (adds folded into phase 3) ----
            for q, (wv, tabv) in enumerate([(wr, c), (wi, s), (wi, c), (wr, s)]):
                pq = p_pool.tile([128, L], BF16, tag=f"p{pt}{q}")
                TT(pq, wv, tabv, AL.mult)
                planes[(pt, q)] = pq
        # ---- phase 3: y tiles ----
        for ti in range(L // 128):
            tsl = slice(ti * 128, (ti + 1) * 128)
            yp = y_psum.tile([128, H], F32, tag="y")
            first = True
            for pt in range(2):
                for q in range(4):
                    nc.tensor.matmul(yp, lhsT=planes[(pt, q)][:, tsl],
                                     rhs=cw[q][pt], start=first, stop=False)
                    first = False
            o = b * L + ti * 128
            for kh in range(2):
                nc.tensor.matmul(yp, lhsT=uT[kh][:, o:o + 128], rhs=dd[kh],
                                 start=False, stop=(kh == 1))
            ys = y_sb.tile([128, H], F32, tag="ys")
            nc.scalar.copy(ys, yp)
            nc.sync.dma_start(d_y[b, tsl, :], ys)


def _legalize_waits(nc, max_waits=1):
    """walrus caps sem-waits per ISA instruction; spill excess onto
    same-engine NOPs inserted immediately before (engine streams are
    in-order, so the combined wait set is preserved)."""
    import bass_rust
    n_split = 0
    for f in nc.m.functions:
        for blk in f.blocks:
            new = []
            for inst in blk.instructions:
                si = inst.sync_info
                if si is not None and len(si.on_wait) > max_waits:
                    waits = list(si.on_wait)
                    keep = waits[-max_waits:]
                    extra = waits[:-max_waits]
                    k = 0
                    while extra:
                        chunk, extra = extra[:max_waits], extra[max_waits:]
                        nop = mybir.InstNoOp(
                            name=f"{inst.name}-wnop{k}", ins=[], outs=[])
                        nop.engine = inst.engine
                        nop.sync_info = bass_rust.SyncInfo(
                            on_wait=chunk, on_update=[])
                        new.append(nop)
                        k += 1
                    inst.sync_info = bass_rust.SyncInfo(
                        on_wait=keep, on_update=list(si.on_update))
                    n_split += 1
                new.append(inst)
            blk.instructions = new
    return n_split


def _build():
    from contextlib import ExitStack
    nc = bass.Bass()
    d_u = nc.declare_dram_parameter("u", [BL, H, L], BF16, False)
    d_w1 = nc.declare_dram_parameter("bbarT", [2, H, P], BF16, False)
    d_cos = nc.declare_dram_parameter("costab", [2, 128, L], BF16, False)
    d_sin = nc.declare_dram_parameter("sintab", [2, 128, L], BF16, False)
    d_r = nc.declare_dram_parameter("rbctab", [2, 128, L], F32, False)
    d_cw = nc.declare_dram_parameter("cw", [4, 2, 128, H], BF16, False)
    d_dd = nc.declare_dram_parameter("diagd", [H, H], BF16, False)
    d_y = nc.declare_dram_parameter("y", [BL, L, H], F32, True)
    with tile.TileContext(nc) as tc, ExitStack() as ctx:
        _emit(ctx, tc, nc, d_u, d_w1, d_cos, d_sin, d_r, d_cw, d_dd, d_y)
    _legalize_waits(nc)
    return nc


def _host_precompute(Lambda_re, Lambda_im, B, C, D, log_step):
    Lam = Lambda_re.astype(np.complex128) + 1j * Lambda_im.astype(np.complex128)
    Delta = np.exp(log_step[:, 0].astype(np.float64))
    lam_bar = np.exp(Lam * Delta)
    r = np.abs(lam_bar)
    theta = Lambda_im.astype(np.float64) * Delta
    Btil = B[..., 0].astype(np.float64) + 1j * B[..., 1].astype(np.float64)
    Bbar = ((lam_bar - 1.0) / Lam)[:, None] * Btil                 # (P,H)
    bbarT = np.stack([Bbar.real.T, Bbar.imag.T]).astype(BF16_NP)  # (2,H,P)
    tgrid = np.arange(L, dtype=np.float64)
    ang = theta[:, None] * tgrid[None, :]                          # (P,L)
    costab = np.cos(ang).astype(BF16_NP).reshape(2, 128, L)
    sintab = np.sin(ang).astype(BF16_NP).reshape(2, 128, L)
    rcol = np.ascontiguousarray(np.broadcast_to(
        r.astype(np.float32)[:, None], (P, L))).reshape(2, 128, L)
    Ct_re = C[..., 0].astype(np.float64)
    Ct_im = C[..., 1].astype(np.float64)
    W = np.stack([2 * Ct_re.T, -2 * Ct_re.T, -2 * Ct_im.T, -2 * Ct_im.T])
    cw = np.ascontiguousarray(W.reshape(4, 2, 128, H)).astype(BF16_NP)
    diagd = np.diag(D).astype(BF16_NP)
    return dict(bbarT=np.ascontiguousarray(bbarT), costab=costab,
                sintab=sintab, rbctab=rcol, cw=cw, diagd=diagd)


def _in_maps(u, shared):
    maps = []
    for c in range(NCORES):
        m = dict(shared)
        m["u"] = np.ascontiguousarray(
            u[c * BL:(c + 1) * BL].transpose(0, 2, 1)).astype(BF16_NP)
        maps.append(m)
    return maps


def _get_prog():
    global _PROG
    if _PROG is None:
        _PROG = _build()
    return _PROG


def _run(in_maps, **kw):
    nc = _get_prog()
    return run_bass_kernel_spmd(nc, in_maps, list(range(NCORES)), **kw)


def kernel(u, Lambda_re, Lambda_im, B, C, D, log_step):
    u = np.asarray(u, dtype=np.float32)
    shared = _host_precompute(np.asarray(Lambda_re), np.asarray(Lambda_im),
                              np.asarray(B), np.asarray(C), np.asarray(D),
                              np.asarray(log_step))
    res = _run(_in_maps(u, shared))
    y = np.concatenate([np.asarray(res.results[c]["y"]) for c in range(NCORES)],
                       axis=0)
    return y.astype(np.float32)
